# revision 12
# baseline (speedup 1.0000x reference)
# Causal GQA self-attention on 8 TRN2 NeuronCores (Bass/Tile, SPMD).
#
# Sharding: core c -> (batch b = c//4, head-group g = c%4). Each core computes
# q-heads 4g..4g+3 with kv-head g for its batch (attention phase), then an
# 8-rank AllToAll re-shards attention outputs from head-split to token-split:
# core c does the output projection for token rows [256c, 256c+256) of BOTH
# batches with the full Wo. Outputs are pure per-core slices (host concat).
#
# Precision: x/Wq/Wk/Wv/cos/sin/Wo and all attention-side tensors (probs, V,
# AllToAll payload) are bf16; q/k stay fp32 (f32r matmuls for scores); all
# accumulation in fp32 PSUM. Heads run in pairs (two independent chains), the
# attention loop is software-pipelined (dn/av matmuls delayed one iteration
# behind the score/exp front-end so the PE never waits on the ACT/DVE chain),
# score tiles pair into 2-bank [128,1024] PSUM tiles, causal masking is a
# multiplicative 0/1 bf16 mask after Exp, and softmax denominators use DVE
# pair-sums to halve the ones-matmul count. The output projection splits into
# a JTA pass (overlaps the second AllToAll) and a JTB pass joined by DVE adds.
import numpy as np

B, T, C = 2, 2048, 2048
H, KV, HD = 16, 4, 128
NCORES = 8
INV_SQRT_HD = 1.0 / float(np.sqrt(HD))

_cache = {}


def _build(t_len, c_len):
    import concourse.bass as bass
    import concourse.mybir as mybir
    import concourse.tile as tile
    from concourse import bacc
    from concourse.masks import make_identity

    F32 = mybir.dt.float32
    F32R = mybir.dt.float32r
    BF16 = mybir.dt.bfloat16
    AF = mybir.ActivationFunctionType
    MUL = mybir.AluOpType.mult
    ADD = mybir.AluOpType.add

    NT = t_len // 128          # token tiles
    NC_ = c_len // 128         # channel tiles
    NCH = t_len // 512         # 512-wide token chunks
    HL = 4                     # local q heads
    TS = t_len // 8            # per-core token slice for o_proj

    nc = bacc.Bacc("TRN2", target_bir_lowering=False, debug=False,
                   num_devices=NCORES)

    xT_ap = nc.dram_tensor("xT", [c_len, t_len], BF16, kind="ExternalInput").ap()
    wq_ap = nc.dram_tensor("wq", [c_len, 512], BF16, kind="ExternalInput").ap()
    wk_ap = nc.dram_tensor("wk", [c_len, 128], BF16, kind="ExternalInput").ap()
    wv_ap = nc.dram_tensor("wv", [c_len, 128], BF16, kind="ExternalInput").ap()
    wo_ap = nc.dram_tensor("wo", [2048, 2048], BF16, kind="ExternalInput").ap()
    cos_ap = nc.dram_tensor("cosT", [128, t_len], BF16, kind="ExternalInput").ap()
    sin_ap = nc.dram_tensor("sinTs", [128, t_len], BF16, kind="ExternalInput").ap()
    msk_ap = nc.dram_tensor("masks", [128, 4 * 512], BF16, kind="ExternalInput").ap()
    o_aps = [nc.dram_tensor(f"o{bb}", [2048, TS], F32, kind="ExternalOutput").ap()
             for bb in range(2)]
    # AllToAll staging: block j = rows [128j, 128j+128) -> core j; cols are
    # [head-within-pair x token-slice]. 1KB rows keep the collective's DMA
    # packets large.
    a2in_a = nc.dram_tensor("a2in_a", [NCORES * 128, 2 * TS], BF16).ap()
    a2out_a = nc.dram_tensor("a2out_a", [NCORES * 128, 2 * TS], BF16).ap()
    a2in_b = nc.dram_tensor("a2in_b", [NCORES * 128, 2 * TS], BF16).ap()
    a2out_b = nc.dram_tensor("a2out_b", [NCORES * 128, 2 * TS], BF16).ap()

    JTA = [jt for jt in range(16) if jt % 4 < 2]
    JTB = [jt for jt in range(16) if jt % 4 >= 2]

    with tile.TileContext(nc) as tc:
        with tc.tile_pool(name="const", bufs=1) as constp:
            idt = constp.tile([128, 128], BF16)
            make_identity(nc, idt[:, :])
            ones_k = constp.tile([128, 1], BF16)
            nc.vector.memset(ones_k[:, :], 1.0)
            ones_m = constp.tile([1, 128], F32)
            nc.vector.memset(ones_m[:, :], 1.0)

            # masks first: attention chunk 0 is 100% mask-dependent
            mskp = tc.alloc_tile_pool(name="mskp", bufs=1)
            masks = mskp.tile([128, 4 * 512], BF16, name="masks")
            nc.gpsimd.dma_start(out=masks[:, :], in_=msk_ap[:, :])

            with tc.tile_pool(name="act", bufs=1) as pp:
                # persistent activations
                qT = [pp.tile([128, t_len], F32R, tag=f"qT{j}", name=f"qT{j}")
                      for j in range(HL)]
                kT = pp.tile([128, t_len], F32R)
                v_t = [pp.tile([128, 128], BF16, tag=f"v{tt}", name=f"v{tt}")
                       for tt in range(NT)]

                # ------------ phase 1+2: projections + RoPE (x pre-transposed
                # on host; PSUM read directly by DVE for the rotate-half)
                with (
                    tc.tile_pool(name="ph2", bufs=1) as ph2,
                    tc.tile_pool(name="ph2ps", bufs=2, space="PSUM") as ph2ps,
                ):
                    cosT = ph2.tile([128, t_len], BF16)
                    nc.gpsimd.dma_start(out=cosT[:, :], in_=cos_ap[:, :])
                    sinTs = ph2.tile([128, t_len], BF16)
                    nc.gpsimd.dma_start(out=sinTs[:, :], in_=sin_ap[:, :])
                    wq_sb = [ph2.tile([128, 512], BF16, tag=f"wq{ct}", name=f"wq{ct}")
                             for ct in range(NC_)]
                    wk_sb = [ph2.tile([128, 128], BF16, tag=f"wk{ct}", name=f"wk{ct}")
                             for ct in range(NC_)]
                    wv_sb = [ph2.tile([128, 128], BF16, tag=f"wv{ct}", name=f"wv{ct}")
                             for ct in range(NC_)]
                    xs0 = [ph2.tile([128, 512], BF16, tag=f"xs{ct}", name=f"xs0_{ct}",
                                    bufs=2)
                           for ct in range(NC_)]
                    # interleave per ct and spread across the three DMA-capable
                    # queues: the warmup is queue-issue-bound, not HBM-bound
                    for ct in range(NC_):
                        nc.sync.dma_start(out=xs0[ct][:, :],
                                          in_=xT_ap[ct*128:(ct+1)*128, 0:512])
                        nc.scalar.dma_start(out=wq_sb[ct][:, :],
                                            in_=wq_ap[ct*128:(ct+1)*128, :])
                        nc.gpsimd.dma_start(out=wk_sb[ct][:, :],
                                            in_=wk_ap[ct*128:(ct+1)*128, :])
                        nc.gpsimd.dma_start(out=wv_sb[ct][:, :],
                                            in_=wv_ap[ct*128:(ct+1)*128, :])
                    for ch in range(NCH):
                        sl = slice(ch * 512, (ch + 1) * 512)
                        if ch == 0:
                            xs = xs0
                        else:
                            xs = [ph2.tile([128, 512], BF16, tag=f"xs{ct}",
                                           name=f"xs{ct}", bufs=2)
                                  for ct in range(NC_)]
                            for ct in range(NC_):
                                nc.sync.dma_start(out=xs[ct][:, :],
                                                  in_=xT_ap[ct*128:(ct+1)*128, sl])
                        # q heads + k: project, then RoPE (DVE reads PSUM)
                        for u in range(HL + 1):
                            ps_a = ph2ps.tile([128, 512], F32, tag="acc", name="ps_a")
                            for ct in range(NC_):
                                w = wq_sb[ct][:, u*128:(u+1)*128] if u < HL else wk_sb[ct][:, :]
                                nc.tensor.matmul(out=ps_a[:, :], lhsT=w, rhs=xs[ct][:, :],
                                                 start=(ct == 0), stop=(ct == NC_ - 1))
                            t1 = ph2.tile([128, 512], F32, tag="t1", bufs=2, name="t1")
                            nc.vector.tensor_tensor(t1[0:64, :], ps_a[64:128, :],
                                                    sinTs[0:64, sl], MUL)
                            nc.vector.tensor_tensor(t1[64:128, :], ps_a[0:64, :],
                                                    sinTs[64:128, sl], MUL)
                            t2 = ph2.tile([128, 512], F32, tag="t2", bufs=2, name="t2")
                            nc.vector.tensor_tensor(t2[:, :], ps_a[:, :], cosT[:, sl], MUL)
                            dst = qT[u][:, sl] if u < HL else kT[:, sl]
                            nc.vector.tensor_tensor(dst, t1[:, :], t2[:, :], ADD)
                        # v: project, then transpose to token-major bf16
                        ps_a = ph2ps.tile([128, 512], F32, tag="acc", name="ps_av")
                        for ct in range(NC_):
                            nc.tensor.matmul(out=ps_a[:, :], lhsT=wv_sb[ct][:, :],
                                             rhs=xs[ct][:, :],
                                             start=(ct == 0), stop=(ct == NC_ - 1))
                        vraw = ph2.tile([128, 512], BF16, tag="vraw", bufs=2, name="vraw")
                        nc.scalar.activation(vraw[:, :], ps_a[:, :], AF.Copy)
                        for tt4 in range(4):
                            ps_tr = ph2ps.tile([128, 128], BF16, tag="tr", name="ps_trv")
                            nc.tensor.transpose(ps_tr[:, :], vraw[:, tt4*128:(tt4+1)*128],
                                                idt[:, :])
                            nc.scalar.activation(v_t[ch*4+tt4][:, :], ps_tr[:, :], AF.Copy)

                # preload ALL Wo row-tiles (bf16, 8.4MB) on the gpsimd DMA
                # queue so they never head-of-line-block the sync queue
                woap = tc.alloc_tile_pool(name="woa", bufs=1)
                wo_a = {jt: woap.tile([128, 2048], BF16, tag=f"wo{jt}",
                                      name=f"wo{jt}") for jt in range(16)}
                for jt in range(16):
                    nc.gpsimd.dma_start(out=wo_a[jt][:, :],
                                        in_=wo_ap[jt*128:(jt+1)*128, :])

                # ---------------- phase 3: attention, head pairs x chunks,
                # software-pipelined: the dn/av matmuls for iteration i are
                # emitted after iteration i+1's scores so the PE queue never
                # blocks on the Exp->mask->pair-sum chain.
                rhsp = tc.alloc_tile_pool(name="rhsp", bufs=1)
                rhs_t = {}
                with (
                    tc.tile_pool(name="ph3", bufs=1) as ph3,
                    tc.tile_pool(name="ph3ps", bufs=1, space="PSUM") as ph3ps,
                ):
                    for hp in (0, 2):
                        pend_be = None
                        pend_fin = None
                        state = {}

                        def frontend(ch, p, np_):
                            sl = slice(ch * 512, (ch + 1) * 512)
                            j0, j1 = 2 * p, 2 * p + 1
                            q, half = p // 2, p % 2
                            nq = np_ // 2
                            if p == 0:
                                state['av'] = [ph3ps.tile([128, 512], F32,
                                                          tag=f"av{hh}",
                                                          name=f"ps_av{hh}")
                                               for hh in (0, 1)]
                                state['dn'] = ph3ps.tile([128, 512], F32,
                                                         tag="den", name="ps_dn")
                            if half == 0:
                                state['aq'] = {hh: ph3.tile([128, 2048], BF16,
                                                            tag=f"a{hh}{q % 2}",
                                                            bufs=2, name=f"aq{hh}")
                                               for hh in (0, 1)}
                            ps_av, ps_dn = state['av'], state['dn']
                            aq = state['aq']
                            ps_sp, qsums = {}, {}
                            for hh in (0, 1):
                                s_ = ph3ps.tile([128, 1024], F32, tag=f"s{hh}",
                                                name=f"ps_s{hh}")
                                ps_sp[hh] = s_
                                nc.tensor.matmul(out=s_[:, 0:512],
                                                 lhsT=kT[:, j0*128:(j0+1)*128],
                                                 rhs=qT[hp+hh][:, sl],
                                                 start=True, stop=True)
                                nc.tensor.matmul(out=s_[:, 512:1024],
                                                 lhsT=kT[:, j1*128:(j1+1)*128],
                                                 rhs=qT[hp+hh][:, sl],
                                                 start=True, stop=True)
                            for hh in (0, 1):
                                nc.scalar.activation(
                                    aq[hh][:, half*1024:(half+1)*1024],
                                    ps_sp[hh][:, :], AF.Exp, scale=INV_SQRT_HD)
                            if half == 1:
                                if q == ch:  # diagonal quad: 0/1 mask
                                    for hh in (0, 1):
                                        nc.vector.tensor_tensor(
                                            aq[hh][:, :], aq[hh][:, :],
                                            masks[:, :], MUL)
                                # quad-sum for the denominator: Pool halves-add
                                # then a DVE add, one ones-matmul per 4 tiles
                                for hh in (0, 1):
                                    asum = ph3.tile([128, 1024], BF16,
                                                    tag=f"as{hh}", bufs=2,
                                                    name=f"asum{hh}")
                                    nc.gpsimd.tensor_tensor(asum[:, :],
                                                            aq[hh][:, 0:1024],
                                                            aq[hh][:, 1024:2048],
                                                            ADD)
                                    qs = ph3.tile([128, 512], BF16,
                                                  tag=f"qs{hh}", bufs=2,
                                                  name=f"qs{hh}")
                                    qsums[hh] = qs
                                    nc.vector.tensor_tensor(qs[:, :],
                                                            asum[:, 0:512],
                                                            asum[:, 512:1024], ADD)

                            def backend():
                                for hh in (0, 1):
                                    if half == 1:
                                        nc.tensor.matmul(out=ps_dn[32*hh:32*hh+1, :],
                                                         lhsT=ones_k[:, :],
                                                         rhs=qsums[hh][:, :],
                                                         start=(q == 0),
                                                         stop=(q == nq - 1))
                                    nc.tensor.matmul(out=ps_av[hh][:, :],
                                                     lhsT=v_t[j0][:, :],
                                                     rhs=aq[hh][:, half*1024:half*1024+512],
                                                     start=(p == 0), stop=False)
                                    nc.tensor.matmul(out=ps_av[hh][:, :],
                                                     lhsT=v_t[j1][:, :],
                                                     rhs=aq[hh][:, half*1024+512:half*1024+1024],
                                                     start=False,
                                                     stop=(p == np_ - 1))
                            return backend

                        def finalize(ch, ps_av, ps_dn, hp=hp):
                            def run():
                                dst = a2in_a if hp == 0 else a2in_b
                                for hh in (0, 1):
                                    dn = ph3.tile([1, 512], F32R, tag=f"dn{hh}",
                                                  bufs=2, name=f"dn{hh}")
                                    nc.vector.tensor_copy(dn[:, :],
                                                          ps_dn[32*hh:32*hh+1, :])
                                    ps_bc = ph3ps.tile([128, 512], F32, tag="bc",
                                                       name="ps_bc")
                                    nc.tensor.matmul(out=ps_bc[:, :],
                                                     lhsT=ones_m[:, :].bitcast(F32R),
                                                     rhs=dn[:, :],
                                                     start=True, stop=True)
                                    rec = ph3.tile([128, 512], F32, tag=f"rec{hh}",
                                                   bufs=2, name=f"rec{hh}")
                                    nc.vector.reciprocal_approx_fast(rec[:, :],
                                                                     ps_bc[:, :])
                                    ao = ph3.tile([128, 512], BF16, tag=f"ao{hh}",
                                                  bufs=4, name=f"ao{hh}")
                                    nc.vector.tensor_tensor(ao[:, :], ps_av[hh][:, :],
                                                            rec[:, :], MUL)
                                    for half in range(512 // TS):
                                        tb = (512 // TS) * ch + half
                                        nc.sync.dma_start(
                                            out=dst[tb*128:(tb+1)*128,
                                                    hh*TS:(hh+1)*TS],
                                            in_=ao[:, half*TS:(half+1)*TS])
                            return run

                        for ch in range(NCH):
                            njt = 4 * ch + 4
                            np_ = njt // 2
                            for p in range(np_):
                                be = frontend(ch, p, np_)
                                if pend_be is not None:
                                    pend_be()
                                if pend_fin is not None:
                                    pend_fin()
                                    pend_fin = None
                                pend_be = be
                                if p == np_ - 1:
                                    pend_fin = finalize(ch, state['av'], state['dn'])
                        pend_be()
                        pend_fin()

                        # AllToAll for this head pair; rhs tiles loaded on the
                        # sync queue as soon as the exchange lands
                        if hp == 0:
                            nc.gpsimd.collective_compute(
                                "AllToAll", mybir.AluOpType.bypass,
                                replica_groups=[list(range(NCORES))],
                                ins=[a2in_a[:, :]], outs=[a2out_a[:, :]],
                            )
                            for jt in JTA:
                                t_ = rhsp.tile([128, 2 * TS], BF16, tag=f"rhs{jt}",
                                               name=f"rhs{jt}")
                                rhs_t[jt] = t_
                                for bb in range(2):
                                    i_ = 4 * bb + jt // 4
                                    hh_ = jt % 4
                                    nc.sync.dma_start(
                                        out=t_[:, bb*TS:(bb+1)*TS],
                                        in_=a2out_a[i_*128:(i_+1)*128,
                                                    hh_*TS:(hh_+1)*TS])
                        else:
                            nc.gpsimd.collective_compute(
                                "AllToAll", mybir.AluOpType.bypass,
                                replica_groups=[list(range(NCORES))],
                                ins=[a2in_b[:, :]], outs=[a2out_b[:, :]],
                            )
                            for jt in JTB:
                                t_ = rhsp.tile([128, 2 * TS], BF16, tag=f"rhs{jt}",
                                               name=f"rhs{jt}")
                                rhs_t[jt] = t_
                                for bb in range(2):
                                    i_ = 4 * bb + jt // 4
                                    hh_ = jt % 4 - 2
                                    nc.sync.dma_start(
                                        out=t_[:, bb*TS:(bb+1)*TS],
                                        in_=a2out_b[i_*128:(i_+1)*128,
                                                    hh_*TS:(hh_+1)*TS])

                # -------- phase 4: o_proj. Pass A (JTA) overlaps the second
                # AllToAll; pass B (JTB) accumulates onto it via DVE adds.
                with (
                    tc.tile_pool(name="ph4", bufs=1) as ph4,
                    tc.tile_pool(name="ph4ps", bufs=2, space="PSUM") as ph4ps,
                ):
                    accA = [ph4.tile([128, 2 * TS], F32, tag=f"accA{cc}",
                                     name=f"accA{cc}") for cc in range(16)]
                    for cc in range(16):
                        ps_o = ph4ps.tile([128, 2 * TS], F32, tag="o", name="ps_o")
                        for idx, jt in enumerate(JTA):
                            nc.tensor.matmul(out=ps_o[:, :],
                                             lhsT=wo_a[jt][:, cc*128:(cc+1)*128],
                                             rhs=rhs_t[jt][:, :],
                                             start=(idx == 0), stop=(idx == 7))
                        nc.scalar.activation(accA[cc][:, :], ps_o[:, :], AF.Copy)
                    for cc in range(16):
                        ps_o = ph4ps.tile([128, 2 * TS], F32, tag="o", name="ps_ob")
                        for idx, jt in enumerate(JTB):
                            nc.tensor.matmul(out=ps_o[:, :],
                                             lhsT=wo_a[jt][:, cc*128:(cc+1)*128],
                                             rhs=rhs_t[jt][:, :],
                                             start=(idx == 0), stop=(idx == 7))
                        osb = ph4.tile([128, 2 * TS], F32, tag="osb", bufs=2,
                                       name="osb")
                        nc.vector.tensor_tensor(osb[:, :], accA[cc][:, :],
                                                ps_o[:, :], ADD)
                        for bb in range(2):
                            nc.sync.dma_start(out=o_aps[bb][cc*128:(cc+1)*128, :],
                                              in_=osb[:, bb*TS:(bb+1)*TS])
                rhsp.release()
                woap.release()
            mskp.release()

    nc.compile()
    return nc


def _to_bf16(a):
    import ml_dtypes
    return np.asarray(a, dtype=np.float32).astype(ml_dtypes.bfloat16)


def _prep_inputs(x, cos, sin, Wq, Wk, Wv, Wo):
    x = np.asarray(x, dtype=np.float32)
    cos = np.asarray(cos, dtype=np.float32)
    sin = np.asarray(sin, dtype=np.float32)

    t_len = x.shape[1]
    cosT = _to_bf16(np.ascontiguousarray(cos.T))             # [128, T]
    sinT = np.ascontiguousarray(sin.T)
    sinTs = sinT.copy()
    sinTs[0:64, :] *= -1.0                                   # signed swap-half
    sinTs = _to_bf16(sinTs)

    Wq_bf = np.ascontiguousarray(_to_bf16(Wq))
    Wk_bf = np.ascontiguousarray(_to_bf16(Wk))
    Wv_bf = np.ascontiguousarray(_to_bf16(Wv))
    Wo_bf = np.ascontiguousarray(_to_bf16(Wo))

    # multiplicative 0/1 causal masks, [128 k-local, 4 diag-offsets x 512 q]
    tk = np.arange(128)[:, None]
    tq = np.arange(512)[None, :]
    masks = np.zeros((128, 4, 512), dtype=np.float32)
    for jd in range(4):
        masks[:, jd, :] = (128 * jd + tk <= tq).astype(np.float32)
    masks = np.ascontiguousarray(_to_bf16(masks.reshape(128, 4 * 512)))

    in_maps = []
    for c in range(NCORES):
        b, g = c // 4, c % 4
        xb = x[b] if x.ndim == 3 else x
        in_maps.append({
            "xT": np.ascontiguousarray(_to_bf16(xb.T)),
            "wq": np.ascontiguousarray(Wq_bf[:, 512*g:512*(g+1)]),
            "wk": np.ascontiguousarray(Wk_bf[:, 128*g:128*(g+1)]),
            "wv": np.ascontiguousarray(Wv_bf[:, 128*g:128*(g+1)]),
            "wo": Wo_bf,
            "cosT": cosT,
            "sinTs": sinTs,
            "masks": masks,
        })
    return in_maps, t_len


def kernel(x, cos, sin, Wq, Wk, Wv, Wo):
    from concourse.bass_utils import run_bass_kernel_spmd

    in_maps, t_len = _prep_inputs(x, cos, sin, Wq, Wk, Wv, Wo)
    c_len = in_maps[0]["xT"].shape[0]
    key = (t_len, c_len)
    if key not in _cache:
        _cache[key] = _build(t_len, c_len)
    nc = _cache[key]

    res = run_bass_kernel_spmd(nc, in_maps, core_ids=list(range(NCORES)))
    ts = t_len // 8
    out = np.empty((2, t_len, 2048), dtype=np.float32)
    for c in range(NCORES):
        out[0, ts*c:ts*(c+1), :] = res.results[c]["o0"].T
        out[1, ts*c:ts*(c+1), :] = res.results[c]["o1"].T
    return out


# revision 13
# speedup vs baseline: 1.4012x; 1.4012x over previous
# Causal GQA self-attention on 8 TRN2 NeuronCores (Bass/Tile, SPMD).
#
# Sharding: core c -> (batch b = c//4, head-group g = c%4). Each core computes
# q-heads 4g..4g+3 with kv-head g for its batch (attention phase), then an
# 8-rank AllToAll re-shards attention outputs from head-split to token-split:
# core c does the output projection for token rows [256c, 256c+256) of BOTH
# batches with the full Wo. Outputs are pure per-core slices (host concat).
#
# Precision: x/Wq/Wk/Wv/cos/sin/Wo and all attention-side tensors (probs, V,
# AllToAll payload) are bf16; q/k stay fp32 (f32r matmuls for scores); all
# accumulation in fp32 PSUM. Heads run in pairs (two independent chains), the
# attention loop is software-pipelined (dn/av matmuls delayed one iteration
# behind the score/exp front-end so the PE never waits on the ACT/DVE chain),
# score tiles pair into 2-bank [128,1024] PSUM tiles, causal masking is a
# multiplicative 0/1 bf16 mask after Exp, and softmax denominators use DVE
# pair-sums to halve the ones-matmul count. The output projection splits into
# a JTA pass (overlaps the second AllToAll) and a JTB pass joined by DVE adds.
import numpy as np

B, T, C = 2, 2048, 2048
H, KV, HD = 16, 4, 128
NCORES = 8
INV_SQRT_HD = 1.0 / float(np.sqrt(HD))

_cache = {}


def _build(t_len, c_len):
    import concourse.bass as bass
    import concourse.mybir as mybir
    import concourse.tile as tile
    from concourse import bacc
    from concourse.masks import make_identity

    F32 = mybir.dt.float32
    F32R = mybir.dt.float32r
    BF16 = mybir.dt.bfloat16
    AF = mybir.ActivationFunctionType
    MUL = mybir.AluOpType.mult
    ADD = mybir.AluOpType.add

    NT = t_len // 128          # token tiles
    NC_ = c_len // 128         # channel tiles
    NCH = t_len // 512         # 512-wide token chunks
    HL = 4                     # local q heads
    TS = t_len // 8            # per-core token slice for o_proj

    nc = bacc.Bacc("TRN2", target_bir_lowering=False, debug=False,
                   num_devices=NCORES)

    xT_ap = nc.dram_tensor("xT", [c_len, t_len], BF16, kind="ExternalInput").ap()
    wq_ap = nc.dram_tensor("wq", [c_len, 512], BF16, kind="ExternalInput").ap()
    wk_ap = nc.dram_tensor("wk", [c_len, 128], BF16, kind="ExternalInput").ap()
    wv_ap = nc.dram_tensor("wv", [c_len, 128], BF16, kind="ExternalInput").ap()
    wo_ap = nc.dram_tensor("wo", [2048, 2048], BF16, kind="ExternalInput").ap()
    cos_ap = nc.dram_tensor("cosT", [128, t_len], BF16, kind="ExternalInput").ap()
    sin_ap = nc.dram_tensor("sinTs", [128, t_len], BF16, kind="ExternalInput").ap()
    msk_ap = nc.dram_tensor("masks", [128, 4 * 512], BF16, kind="ExternalInput").ap()
    o_aps = [nc.dram_tensor(f"o{bb}", [2048, TS], F32, kind="ExternalOutput").ap()
             for bb in range(2)]
    # AllToAll staging: block j = rows [128j, 128j+128) -> core j; cols are
    # [head-within-pair x token-slice]. 1KB rows keep the collective's DMA
    # packets large.
    a2in_a = nc.dram_tensor("a2in_a", [NCORES * 128, 2 * TS], BF16).ap()
    a2out_a = nc.dram_tensor("a2out_a", [NCORES * 128, 2 * TS], BF16).ap()
    a2in_b = nc.dram_tensor("a2in_b", [NCORES * 128, 2 * TS], BF16).ap()
    a2out_b = nc.dram_tensor("a2out_b", [NCORES * 128, 2 * TS], BF16).ap()

    JTA = [jt for jt in range(16) if jt % 4 < 2]
    JTB = [jt for jt in range(16) if jt % 4 >= 2]

    with tile.TileContext(nc) as tc:
        with tc.tile_pool(name="const", bufs=1) as constp:
            idt = constp.tile([128, 128], BF16)
            make_identity(nc, idt[:, :])
            ones_k = constp.tile([128, 1], BF16)
            nc.vector.memset(ones_k[:, :], 1.0)
            ones_m = constp.tile([1, 128], F32)
            nc.vector.memset(ones_m[:, :], 1.0)

            # masks first: attention chunk 0 is 100% mask-dependent
            mskp = tc.alloc_tile_pool(name="mskp", bufs=1)
            masks = mskp.tile([128, 4 * 512], BF16, name="masks")
            nc.gpsimd.dma_start(out=masks[:, :], in_=msk_ap[:, :])

            with tc.tile_pool(name="act", bufs=1) as pp:
                # persistent activations
                qT = [pp.tile([128, t_len], F32R, tag=f"qT{j}", name=f"qT{j}")
                      for j in range(HL)]
                kT = pp.tile([128, t_len], F32R)
                v_t = [pp.tile([128, 128], BF16, tag=f"v{tt}", name=f"v{tt}")
                       for tt in range(NT)]

                # ------------ phase 1+2: projections + RoPE (x pre-transposed
                # on host; PSUM read directly by DVE for the rotate-half)
                with (
                    tc.tile_pool(name="ph2", bufs=1) as ph2,
                    tc.tile_pool(name="ph2ps", bufs=2, space="PSUM") as ph2ps,
                ):
                    cosT = ph2.tile([128, t_len], BF16)
                    nc.gpsimd.dma_start(out=cosT[:, :], in_=cos_ap[:, :])
                    sinTs = ph2.tile([128, t_len], BF16)
                    nc.gpsimd.dma_start(out=sinTs[:, :], in_=sin_ap[:, :])
                    wq_sb = [ph2.tile([128, 512], BF16, tag=f"wq{ct}", name=f"wq{ct}")
                             for ct in range(NC_)]
                    wk_sb = [ph2.tile([128, 128], BF16, tag=f"wk{ct}", name=f"wk{ct}")
                             for ct in range(NC_)]
                    wv_sb = [ph2.tile([128, 128], BF16, tag=f"wv{ct}", name=f"wv{ct}")
                             for ct in range(NC_)]
                    xs0 = [ph2.tile([128, 512], BF16, tag=f"xs{ct}", name=f"xs0_{ct}",
                                    bufs=2)
                           for ct in range(NC_)]
                    # interleave per ct and spread across the three DMA-capable
                    # queues: the warmup is queue-issue-bound, not HBM-bound
                    for ct in range(NC_):
                        nc.sync.dma_start(out=xs0[ct][:, :],
                                          in_=xT_ap[ct*128:(ct+1)*128, 0:512])
                        nc.scalar.dma_start(out=wq_sb[ct][:, :],
                                            in_=wq_ap[ct*128:(ct+1)*128, :])
                        nc.gpsimd.dma_start(out=wk_sb[ct][:, :],
                                            in_=wk_ap[ct*128:(ct+1)*128, :])
                        nc.gpsimd.dma_start(out=wv_sb[ct][:, :],
                                            in_=wv_ap[ct*128:(ct+1)*128, :])
                    for ch in range(NCH):
                        sl = slice(ch * 512, (ch + 1) * 512)
                        if ch == 0:
                            xs = xs0
                        else:
                            xs = [ph2.tile([128, 512], BF16, tag=f"xs{ct}",
                                           name=f"xs{ct}", bufs=2)
                                  for ct in range(NC_)]
                            for ct in range(NC_):
                                nc.sync.dma_start(out=xs[ct][:, :],
                                                  in_=xT_ap[ct*128:(ct+1)*128, sl])
                        # q heads + k: project, then RoPE (DVE reads PSUM)
                        for u in range(HL + 1):
                            ps_a = ph2ps.tile([128, 512], F32, tag="acc", name="ps_a")
                            for ct in range(NC_):
                                w = wq_sb[ct][:, u*128:(u+1)*128] if u < HL else wk_sb[ct][:, :]
                                nc.tensor.matmul(out=ps_a[:, :], lhsT=w, rhs=xs[ct][:, :],
                                                 start=(ct == 0), stop=(ct == NC_ - 1))
                            t1 = ph2.tile([128, 512], F32, tag="t1", bufs=2, name="t1")
                            nc.vector.tensor_tensor(t1[0:64, :], ps_a[64:128, :],
                                                    sinTs[0:64, sl], MUL)
                            nc.vector.tensor_tensor(t1[64:128, :], ps_a[0:64, :],
                                                    sinTs[64:128, sl], MUL)
                            t2 = ph2.tile([128, 512], F32, tag="t2", bufs=2, name="t2")
                            nc.vector.tensor_tensor(t2[:, :], ps_a[:, :], cosT[:, sl], MUL)
                            dst = qT[u][:, sl] if u < HL else kT[:, sl]
                            nc.vector.tensor_tensor(dst, t1[:, :], t2[:, :], ADD)
                        # v: project, then transpose to token-major bf16
                        ps_a = ph2ps.tile([128, 512], F32, tag="acc", name="ps_av")
                        for ct in range(NC_):
                            nc.tensor.matmul(out=ps_a[:, :], lhsT=wv_sb[ct][:, :],
                                             rhs=xs[ct][:, :],
                                             start=(ct == 0), stop=(ct == NC_ - 1))
                        vraw = ph2.tile([128, 512], BF16, tag="vraw", bufs=2, name="vraw")
                        nc.scalar.activation(vraw[:, :], ps_a[:, :], AF.Copy)
                        for tt4 in range(4):
                            ps_tr = ph2ps.tile([128, 128], BF16, tag="tr", name="ps_trv")
                            nc.tensor.transpose(ps_tr[:, :], vraw[:, tt4*128:(tt4+1)*128],
                                                idt[:, :])
                            nc.scalar.activation(v_t[ch*4+tt4][:, :], ps_tr[:, :], AF.Copy)

                # preload ALL Wo row-tiles (bf16, 8.4MB) on the gpsimd DMA
                # queue so they never head-of-line-block the sync queue
                woap = tc.alloc_tile_pool(name="woa", bufs=1)
                wo_a = {jt: woap.tile([128, 2048], BF16, tag=f"wo{jt}",
                                      name=f"wo{jt}") for jt in range(16)}
                for jt in range(16):
                    nc.gpsimd.dma_start(out=wo_a[jt][:, :],
                                        in_=wo_ap[jt*128:(jt+1)*128, :])

                # ---------------- phase 3: attention, head pairs x chunks,
                # software-pipelined: the dn/av matmuls for iteration i are
                # emitted after iteration i+1's scores so the PE queue never
                # blocks on the Exp->mask->pair-sum chain.
                rhsp = tc.alloc_tile_pool(name="rhsp", bufs=1)
                rhs_t = {}
                with (
                    tc.tile_pool(name="ph3", bufs=1) as ph3,
                    tc.tile_pool(name="ph3ps", bufs=1, space="PSUM") as ph3ps,
                ):
                    for hp in (0, 2):
                        pend_be = None
                        pend_fin = None
                        state = {}

                        def frontend(ch, p, np_):
                            sl = slice(ch * 512, (ch + 1) * 512)
                            j0, j1 = 2 * p, 2 * p + 1
                            q, half = p // 2, p % 2
                            nq = np_ // 2
                            if p == 0:
                                state['av'] = [ph3ps.tile([128, 512], F32,
                                                          tag=f"av{hh}",
                                                          name=f"ps_av{hh}")
                                               for hh in (0, 1)]
                                state['dn'] = ph3ps.tile([128, 512], F32,
                                                         tag="den", name="ps_dn")
                            if half == 0:
                                state['aq'] = {hh: ph3.tile([128, 2048], BF16,
                                                            tag=f"a{hh}{q % 2}",
                                                            bufs=2, name=f"aq{hh}")
                                               for hh in (0, 1)}
                            ps_av, ps_dn = state['av'], state['dn']
                            aq = state['aq']
                            ps_sp, qsums = {}, {}
                            for hh in (0, 1):
                                s_ = ph3ps.tile([128, 1024], F32, tag=f"s{hh}",
                                                name=f"ps_s{hh}")
                                ps_sp[hh] = s_
                                nc.tensor.matmul(out=s_[:, 0:512],
                                                 lhsT=kT[:, j0*128:(j0+1)*128],
                                                 rhs=qT[hp+hh][:, sl],
                                                 start=True, stop=True)
                                nc.tensor.matmul(out=s_[:, 512:1024],
                                                 lhsT=kT[:, j1*128:(j1+1)*128],
                                                 rhs=qT[hp+hh][:, sl],
                                                 start=True, stop=True)
                            for hh in (0, 1):
                                nc.scalar.activation(
                                    aq[hh][:, half*1024:(half+1)*1024],
                                    ps_sp[hh][:, :], AF.Exp, scale=INV_SQRT_HD)
                            if half == 1:
                                if q == ch:  # diagonal quad: 0/1 mask
                                    for hh in (0, 1):
                                        nc.vector.tensor_tensor(
                                            aq[hh][:, :], aq[hh][:, :],
                                            masks[:, :], MUL)
                                # quad-sum for the denominator: two DVE adds,
                                # one ones-matmul per 4 tiles (gpsimd must stay
                                # idle -- collectives execute there)
                                for hh in (0, 1):
                                    asum = ph3.tile([128, 1024], BF16,
                                                    tag=f"as{hh}", bufs=2,
                                                    name=f"asum{hh}")
                                    nc.vector.tensor_tensor(asum[:, :],
                                                            aq[hh][:, 0:1024],
                                                            aq[hh][:, 1024:2048],
                                                            ADD)
                                    qs = ph3.tile([128, 512], BF16,
                                                  tag=f"qs{hh}", bufs=2,
                                                  name=f"qs{hh}")
                                    qsums[hh] = qs
                                    nc.vector.tensor_tensor(qs[:, :],
                                                            asum[:, 0:512],
                                                            asum[:, 512:1024], ADD)

                            def backend():
                                for hh in (0, 1):
                                    if half == 1:
                                        nc.tensor.matmul(out=ps_dn[32*hh:32*hh+1, :],
                                                         lhsT=ones_k[:, :],
                                                         rhs=qsums[hh][:, :],
                                                         start=(q == 0),
                                                         stop=(q == nq - 1))
                                    nc.tensor.matmul(out=ps_av[hh][:, :],
                                                     lhsT=v_t[j0][:, :],
                                                     rhs=aq[hh][:, half*1024:half*1024+512],
                                                     start=(p == 0), stop=False)
                                    nc.tensor.matmul(out=ps_av[hh][:, :],
                                                     lhsT=v_t[j1][:, :],
                                                     rhs=aq[hh][:, half*1024+512:half*1024+1024],
                                                     start=False,
                                                     stop=(p == np_ - 1))
                            return backend

                        def finalize(ch, ps_av, ps_dn, hp=hp):
                            def run():
                                dst = a2in_a if hp == 0 else a2in_b
                                for hh in (0, 1):
                                    dn = ph3.tile([1, 512], F32R, tag=f"dn{hh}",
                                                  bufs=2, name=f"dn{hh}")
                                    nc.vector.tensor_copy(dn[:, :],
                                                          ps_dn[32*hh:32*hh+1, :])
                                    ps_bc = ph3ps.tile([128, 512], F32, tag="bc",
                                                       name="ps_bc")
                                    nc.tensor.matmul(out=ps_bc[:, :],
                                                     lhsT=ones_m[:, :].bitcast(F32R),
                                                     rhs=dn[:, :],
                                                     start=True, stop=True)
                                    rec = ph3.tile([128, 512], F32, tag=f"rec{hh}",
                                                   bufs=2, name=f"rec{hh}")
                                    nc.vector.reciprocal_approx_fast(rec[:, :],
                                                                     ps_bc[:, :])
                                    ao = ph3.tile([128, 512], BF16, tag=f"ao{hh}",
                                                  bufs=4, name=f"ao{hh}")
                                    nc.vector.tensor_tensor(ao[:, :], ps_av[hh][:, :],
                                                            rec[:, :], MUL)
                                    for half in range(512 // TS):
                                        tb = (512 // TS) * ch + half
                                        nc.sync.dma_start(
                                            out=dst[tb*128:(tb+1)*128,
                                                    hh*TS:(hh+1)*TS],
                                            in_=ao[:, half*TS:(half+1)*TS])
                            return run

                        for ch in range(NCH):
                            njt = 4 * ch + 4
                            np_ = njt // 2
                            for p in range(np_):
                                be = frontend(ch, p, np_)
                                if pend_be is not None:
                                    pend_be()
                                if pend_fin is not None:
                                    pend_fin()
                                    pend_fin = None
                                pend_be = be
                                if p == np_ - 1:
                                    pend_fin = finalize(ch, state['av'], state['dn'])
                        pend_be()
                        pend_fin()

                        # AllToAll for this head pair; rhs tiles loaded on the
                        # sync queue as soon as the exchange lands
                        if hp == 0:
                            nc.gpsimd.collective_compute(
                                "AllToAll", mybir.AluOpType.bypass,
                                replica_groups=[list(range(NCORES))],
                                ins=[a2in_a[:, :]], outs=[a2out_a[:, :]],
                            )
                            for jt in JTA:
                                t_ = rhsp.tile([128, 2 * TS], BF16, tag=f"rhs{jt}",
                                               name=f"rhs{jt}")
                                rhs_t[jt] = t_
                                for bb in range(2):
                                    i_ = 4 * bb + jt // 4
                                    hh_ = jt % 4
                                    nc.sync.dma_start(
                                        out=t_[:, bb*TS:(bb+1)*TS],
                                        in_=a2out_a[i_*128:(i_+1)*128,
                                                    hh_*TS:(hh_+1)*TS])
                        else:
                            nc.gpsimd.collective_compute(
                                "AllToAll", mybir.AluOpType.bypass,
                                replica_groups=[list(range(NCORES))],
                                ins=[a2in_b[:, :]], outs=[a2out_b[:, :]],
                            )
                            for jt in JTB:
                                t_ = rhsp.tile([128, 2 * TS], BF16, tag=f"rhs{jt}",
                                               name=f"rhs{jt}")
                                rhs_t[jt] = t_
                                for bb in range(2):
                                    i_ = 4 * bb + jt // 4
                                    hh_ = jt % 4 - 2
                                    nc.sync.dma_start(
                                        out=t_[:, bb*TS:(bb+1)*TS],
                                        in_=a2out_b[i_*128:(i_+1)*128,
                                                    hh_*TS:(hh_+1)*TS])

                # -------- phase 4: o_proj. Pass A (JTA) overlaps the second
                # AllToAll; pass B (JTB) accumulates onto it via DVE adds.
                with (
                    tc.tile_pool(name="ph4", bufs=1) as ph4,
                    tc.tile_pool(name="ph4ps", bufs=2, space="PSUM") as ph4ps,
                ):
                    accA = [ph4.tile([128, 2 * TS], F32, tag=f"accA{cc}",
                                     name=f"accA{cc}") for cc in range(16)]
                    for cc in range(16):
                        ps_o = ph4ps.tile([128, 2 * TS], F32, tag="o", name="ps_o")
                        for idx, jt in enumerate(JTA):
                            nc.tensor.matmul(out=ps_o[:, :],
                                             lhsT=wo_a[jt][:, cc*128:(cc+1)*128],
                                             rhs=rhs_t[jt][:, :],
                                             start=(idx == 0), stop=(idx == 7))
                        nc.scalar.activation(accA[cc][:, :], ps_o[:, :], AF.Copy)
                    for cc in range(16):
                        ps_o = ph4ps.tile([128, 2 * TS], F32, tag="o", name="ps_ob")
                        for idx, jt in enumerate(JTB):
                            nc.tensor.matmul(out=ps_o[:, :],
                                             lhsT=wo_a[jt][:, cc*128:(cc+1)*128],
                                             rhs=rhs_t[jt][:, :],
                                             start=(idx == 0), stop=(idx == 7))
                        osb = ph4.tile([128, 2 * TS], F32, tag="osb", bufs=2,
                                       name="osb")
                        nc.vector.tensor_tensor(osb[:, :], accA[cc][:, :],
                                                ps_o[:, :], ADD)
                        for bb in range(2):
                            nc.sync.dma_start(out=o_aps[bb][cc*128:(cc+1)*128, :],
                                              in_=osb[:, bb*TS:(bb+1)*TS])
                rhsp.release()
                woap.release()
            mskp.release()

    nc.compile()
    return nc


def _to_bf16(a):
    import ml_dtypes
    return np.asarray(a, dtype=np.float32).astype(ml_dtypes.bfloat16)


def _prep_inputs(x, cos, sin, Wq, Wk, Wv, Wo):
    x = np.asarray(x, dtype=np.float32)
    cos = np.asarray(cos, dtype=np.float32)
    sin = np.asarray(sin, dtype=np.float32)

    t_len = x.shape[1]
    cosT = _to_bf16(np.ascontiguousarray(cos.T))             # [128, T]
    sinT = np.ascontiguousarray(sin.T)
    sinTs = sinT.copy()
    sinTs[0:64, :] *= -1.0                                   # signed swap-half
    sinTs = _to_bf16(sinTs)

    Wq_bf = np.ascontiguousarray(_to_bf16(Wq))
    Wk_bf = np.ascontiguousarray(_to_bf16(Wk))
    Wv_bf = np.ascontiguousarray(_to_bf16(Wv))
    Wo_bf = np.ascontiguousarray(_to_bf16(Wo))

    # multiplicative 0/1 causal masks, [128 k-local, 4 diag-offsets x 512 q]
    tk = np.arange(128)[:, None]
    tq = np.arange(512)[None, :]
    masks = np.zeros((128, 4, 512), dtype=np.float32)
    for jd in range(4):
        masks[:, jd, :] = (128 * jd + tk <= tq).astype(np.float32)
    masks = np.ascontiguousarray(_to_bf16(masks.reshape(128, 4 * 512)))

    in_maps = []
    for c in range(NCORES):
        b, g = c // 4, c % 4
        xb = x[b] if x.ndim == 3 else x
        in_maps.append({
            "xT": np.ascontiguousarray(_to_bf16(xb.T)),
            "wq": np.ascontiguousarray(Wq_bf[:, 512*g:512*(g+1)]),
            "wk": np.ascontiguousarray(Wk_bf[:, 128*g:128*(g+1)]),
            "wv": np.ascontiguousarray(Wv_bf[:, 128*g:128*(g+1)]),
            "wo": Wo_bf,
            "cosT": cosT,
            "sinTs": sinTs,
            "masks": masks,
        })
    return in_maps, t_len


def kernel(x, cos, sin, Wq, Wk, Wv, Wo):
    from concourse.bass_utils import run_bass_kernel_spmd

    in_maps, t_len = _prep_inputs(x, cos, sin, Wq, Wk, Wv, Wo)
    c_len = in_maps[0]["xT"].shape[0]
    key = (t_len, c_len)
    if key not in _cache:
        _cache[key] = _build(t_len, c_len)
    nc = _cache[key]

    res = run_bass_kernel_spmd(nc, in_maps, core_ids=list(range(NCORES)))
    ts = t_len // 8
    out = np.empty((2, t_len, 2048), dtype=np.float32)
    for c in range(NCORES):
        out[0, ts*c:ts*(c+1), :] = res.results[c]["o0"].T
        out[1, ts*c:ts*(c+1), :] = res.results[c]["o1"].T
    return out
